# revision 23
# baseline (speedup 1.0000x reference)
"""Trainium2 Bass kernel for nn_Attention_39676907884025.

Reference semantics: q_param (a scalar) is broadcast over both query and key,
so the score matrix qk[b,q,k] = sum_d p*p is CONSTANT along the softmax axis.
Softmax of a constant row is exactly uniform (x - max(x) == 0 bit-exactly,
exp(0) == 1, sum == SK exactly, 1/SK is a power of two), so

    out[b, q, :] = (1/SK) * sum_k value[b, k, :]     for every q.

query / key / q_param never need to touch the device.

Distribution: data-parallel over batch B=16 across 8 NeuronCores (2 batches
per core). Per core and batch:
  1. one 1MB DMA load of value[b] (2048, 128) into SBUF laid out as
     (p=128 partitions, 16*128 free) with k = t*128 + p,
  2. an exact fp32 DVE add-tree folding the 16 k-tiles -> (128, 128),
  3. one fp32 matmul with a constant (1/2048) matrix as stationary weights:
     psum[q, d] = sum_p (1/2048) * acc[p, d] -- reduces across partitions AND
     broadcasts the mean row to all 128 q-partitions in one shot,
  4. 16 x 64KB DMA stores of that tile into the output rows.
"""

import sys

import numpy as np

if "/opt/trn_rl_repo" not in sys.path:
    sys.path.insert(0, "/opt/trn_rl_repo")

B, SQ, SK, D, DV = 16, 2048, 2048, 128, 128
N_CORES = 8
BPC = B // N_CORES  # batches per core
P = 128

LAST_RESULT = None  # BassKernelResults of the most recent run (for profiling)


def _build_nc():
    import concourse.bacc as bacc
    import concourse.mybir as mybir
    from concourse.tile import TileContext

    f32 = mybir.dt.float32
    nc = bacc.Bacc("TRN2", target_bir_lowering=False)

    val = nc.dram_tensor("value", [BPC, SK, DV], f32, kind="ExternalInput")
    out = nc.dram_tensor("out", [BPC, SQ, DV], f32, kind="ExternalOutput")

    nt = SK // P  # 16 k-tiles per batch
    nq = SQ // P  # 16 q-tiles per batch

    with TileContext(nc) as tc:
        with (
            tc.tile_pool(name="x", bufs=2) as xpool,
            tc.tile_pool(name="tree", bufs=2) as tpool,
            tc.tile_pool(name="const", bufs=1) as cpool,
            tc.tile_pool(name="psum", bufs=2, space="PSUM") as ppool,
        ):
            # Stationary matmul operand: every entry 1/SK (exact power of 2),
            # so the partition-reduction matmul also applies the softmax
            # weight exactly.
            w = cpool.tile([P, P], f32)
            nc.vector.memset(w[:], 1.0 / SK)

            # Queue pick per chunk index: even -> Sync HWDGE, odd -> Act HWDGE.
            dma_eng = [nc.sync, nc.scalar]

            # Tiny warm-up DMAs: pay each queue's cold first-byte latency
            # during the startup window rather than on the first real load.
            wu = cpool.tile([1, DV], f32, tag="warm0")
            nc.sync.dma_start(wu[:], val[0, 0:1, :])
            wu2 = cpool.tile([1, DV], f32, tag="warm1")
            nc.scalar.dma_start(wu2[:], val[0, 1:2, :])

            for b in range(BPC):
                # SBUF xt[p, t*128 + d] = value[b, p*16 + t, d]: each
                # partition owns 16 consecutive DRAM rows (8KB contiguous).
                # Load in 4 quarter-chunks (256KB each, alternating HWDGE
                # queues) and reduce each quarter as soon as it lands, so
                # DVE work overlaps the remaining loads.
                xt = xpool.tile([P, SK], f32)
                xdst = xt[:].rearrange("p (t d) -> p t d", d=DV)
                xsrc = val[b].rearrange("(p t) d -> p t d", p=P)

                accs = []
                load_eng = [nc.sync, nc.scalar, nc.gpsimd, nc.sync]
                for qi in range(4):
                    t0, t1 = 4 * qi, 4 * (qi + 1)
                    load_eng[qi].dma_start(
                        xdst[:, t0:t1, :], xsrc[:, t0:t1, :]
                    )
                    lo, hi = 512 * qi, 512 * (qi + 1)
                    a = tpool.tile([P, 256], f32, tag=f"a{qi % 2}")
                    nc.vector.tensor_add(
                        a[:], xt[:, lo : lo + 256], xt[:, lo + 256 : hi]
                    )
                    acc = tpool.tile([P, P], f32, tag=f"acc{qi}")
                    nc.vector.tensor_add(acc[:], a[:, 0:128], a[:, 128:256])
                    accs.append(acc)

                s01 = tpool.tile([P, P], f32, tag="s01")
                nc.vector.tensor_add(s01[:], accs[0][:], accs[1][:])
                s23 = tpool.tile([P, P], f32, tag="s23")
                nc.vector.tensor_add(s23[:], accs[2][:], accs[3][:])
                t4 = tpool.tile([P, P], f32, tag="t4")
                nc.vector.tensor_add(t4[:], s01[:], s23[:])

                # psum[q, d] = sum_p (1/SK) * t4[p, d]  for all q rows.
                ps = ppool.tile([P, P], f32)
                nc.tensor.matmul(ps[:], w[:], t4[:], start=True, stop=True)

                # Replicate the mean tile 4x along the free axis; each of the
                # 4 stores (alternating queues) reads it, covering 4 output
                # row-tiles (256KB) apiece.
                wide = xpool.tile([P, 512], f32, tag="wide")
                nc.vector.tensor_copy(wide[:, 0:P], ps[:])
                nc.vector.tensor_copy(wide[:, P : 2 * P], wide[:, 0:P])
                nc.vector.tensor_copy(wide[:, 2 * P : 4 * P], wide[:, 0 : 2 * P])

                odst = out[b].rearrange("(p t) d -> p t d", p=P)
                wsrc = wide[:].rearrange("p (t d) -> p t d", d=DV)
                for qi in range(4):
                    t0, t1 = 4 * qi, 4 * (qi + 1)
                    dma_eng[qi % 2].dma_start(odst[:, t0:t1, :], wsrc)

    nc.compile()
    return nc


def _build_nc_raw():
    """Raw bacc version (no TileContext): manual semaphores, minimal
    framework overhead. Loads are chunked (4 t-tiles, 256KB apiece, two
    chunks per batch per HWDGE queue) and each chunk is tree-reduced on DVE
    as soon as it lands, so DVE overlaps the remaining loads. The partition
    reduction runs as two PSUM-accumulating (1/SK)-weighted matmuls, then a
    4x replicate and two 256KB stores per queue per batch."""
    import concourse.bacc as bacc
    import concourse.mybir as mybir

    f32 = mybir.dt.float32
    nc = bacc.Bacc("TRN2", target_bir_lowering=False)

    val = nc.dram_tensor("value", [BPC, SK, DV], f32, kind="ExternalInput")
    out = nc.dram_tensor("out", [BPC, SQ, DV], f32, kind="ExternalOutput")

    w = nc.alloc_sbuf_tensor("w_const", [P, P], f32)
    warm = nc.alloc_sbuf_tensor("warm", [P, DV], f32)
    xts = [nc.alloc_sbuf_tensor(f"xt{b}", [P, SK], f32) for b in range(BPC)]
    t1s = [nc.alloc_sbuf_tensor(f"t1_{b}", [P, 1024], f32) for b in range(BPC)]
    t2s = [nc.alloc_sbuf_tensor(f"t2_{b}", [P, 512], f32) for b in range(BPC)]
    t3s = [nc.alloc_sbuf_tensor(f"t3_{b}", [P, 256], f32) for b in range(BPC)]
    t4s = [nc.alloc_sbuf_tensor(f"t4_{b}", [P, P], f32) for b in range(BPC)]
    wides = [nc.alloc_sbuf_tensor(f"wide{b}", [P, 512], f32) for b in range(BPC)]
    psA = [nc.alloc_psum_tensor(f"psA{b}", [P, P], f32) for b in range(BPC)]
    psB = [nc.alloc_psum_tensor(f"psB{b}", [P, P], f32) for b in range(BPC)]

    s_lq1 = nc.alloc_semaphore("s_lq1")
    s_lq2 = nc.alloc_semaphore("s_lq2")
    s_lq3 = nc.alloc_semaphore("s_lq3")
    s_w = nc.alloc_semaphore("s_w")
    s_dve = nc.alloc_semaphore("s_dve")
    s_mm = nc.alloc_semaphore("s_mm")
    s_wide = nc.alloc_semaphore("s_wide")
    s_sq1 = nc.alloc_semaphore("s_sq1")
    s_sq2 = nc.alloc_semaphore("s_sq2")
    s_warm = nc.alloc_semaphore("s_warm")

    def xdst(b):
        return xts[b][:].rearrange("p (t d) -> p t d", d=DV)

    def xsrc(b):
        return val[b].rearrange("(p t) d -> p t d", p=P)

    def odst(b):
        return out[b].rearrange("(p t) d -> p t d", p=P)

    def wsrc(b):
        return wides[b][:].rearrange("p (t d) -> p t d", d=DV)

    with nc.Block() as block:

        @block.sync
        def _(sync):
            # Tiny warm-up DMA so the queue's cold first-byte latency is
            # paid during the startup barrier, not the first real load.
            sync.dma_start(warm[0:1, :], val[0, 0:1, :]).then_inc(s_warm, 16)
            for b in range(BPC):  # front half: t-tiles 0..7 (512KB)
                sync.dma_start(
                    xdst(b)[:, 0:8, :], xsrc(b)[:, 0:8, :]
                ).then_inc(s_lq1, 16)
            # batch-0 front stores
            sync.wait_ge(s_wide, 1)
            sync.dma_start(odst(0)[:, 0:4, :], wsrc(0)).then_inc(s_sq1, 16)
            sync.dma_start(odst(0)[:, 4:8, :], wsrc(0)).then_inc(s_sq1, 16)
            sync.wait_ge(s_sq1, 64)
            sync.wait_ge(s_warm, 16)

        @block.scalar
        def _(scalar):
            # The Act HWDGE queue has a ~3us cold-start lag, so it only gets
            # work whose latency is hidden: batch-1's back-half load and all
            # of batch-1's stores.
            scalar.dma_start(warm[1:2, :], val[0, 1:2, :]).then_inc(s_warm, 16)
            scalar.dma_start(
                xdst(1)[:, 8:16, :], xsrc(1)[:, 8:16, :]
            ).then_inc(s_lq2, 16)
            scalar.wait_ge(s_wide, 2)
            for t0 in (0, 4, 8, 12):
                scalar.dma_start(
                    odst(1)[:, t0 : t0 + 4, :], wsrc(1)
                ).then_inc(s_sq2, 16)
            scalar.wait_ge(s_sq2, 64)
            scalar.wait_ge(s_warm, 32)

        @block.gpsimd
        def _(gpsimd):
            # Third DMA queue (SWDGE): batch-0 back-half load, batch-0 back
            # stores.
            gpsimd.dma_start(
                xdst(0)[:, 8:16, :], xsrc(0)[:, 8:16, :]
            ).then_inc(s_lq3, 16)
            gpsimd.wait_ge(s_wide, 1)
            gpsimd.dma_start(odst(0)[:, 8:12, :], wsrc(0)).then_inc(s_sq1, 16)
            gpsimd.dma_start(odst(0)[:, 12:16, :], wsrc(0)).then_inc(s_sq1, 16)
            gpsimd.wait_ge(s_sq1, 64)

        @block.vector
        def _(vector):
            vector.memset(w[:], 1.0 / SK).then_inc(s_w, 1)
            for b in range(BPC):
                xt, scr = xts[b], t1s[b]
                # front half (cols 0..1023): from sync
                vector.wait_ge(s_lq1, 16 * (b + 1))
                vector.tensor_add(
                    t2s[b][:], xt[:, 0:512], xt[:, 512:1024]
                )
                vector.tensor_add(
                    t3s[b][:], t2s[b][:, 0:256], t2s[b][:, 256:512]
                )
                vector.tensor_add(
                    scr[:, 0:128], t3s[b][:, 0:128], t3s[b][:, 128:256]
                ).then_inc(s_dve, 1)  # accF
                # back half (cols 1024..2047): b0 from gpsimd, b1 from scalar
                vector.wait_ge(s_lq3 if b == 0 else s_lq2, 16)
                vector.tensor_add(
                    scr[:, 128:640], xt[:, 1024:1536], xt[:, 1536:2048]
                )
                vector.tensor_add(
                    scr[:, 640:896], scr[:, 128:384], scr[:, 384:640]
                )
                vector.tensor_add(
                    scr[:, 896:1024], scr[:, 640:768], scr[:, 768:896]
                ).then_inc(s_dve, 1)  # accB
                # wide[0:128] = psA + psB (exact), then replicate to 4 copies.
                # (DVE reads at most one PSUM operand per op, so stage psA
                # into SBUF while mmB is still running.)
                wide = wides[b]
                vector.wait_ge(s_mm, 2 * b + 1)
                vector.tensor_copy(t2s[b][:, 0:P], psA[b][:])
                vector.wait_ge(s_mm, 2 * (b + 1))
                vector.tensor_add(wide[:, 0:P], t2s[b][:, 0:P], psB[b][:])
                vector.tensor_copy(wide[:, P : 2 * P], wide[:, 0:P])
                vector.tensor_copy(
                    wide[:, 2 * P : 4 * P], wide[:, 0 : 2 * P]
                ).then_inc(s_wide, 1)

        @block.tensor
        def _(tensor):
            tensor.wait_ge(s_w, 1)
            for b in range(BPC):
                tensor.wait_ge(s_dve, 2 * b + 1)
                nc.tensor.matmul(
                    psA[b][:], w[:], t1s[b][:, 0:128], start=True, stop=True
                ).then_inc(s_mm, 1)
                tensor.wait_ge(s_dve, 2 * b + 2)
                nc.tensor.matmul(
                    psB[b][:], w[:], t1s[b][:, 896:1024], start=True, stop=True
                ).then_inc(s_mm, 1)

    nc.compile()
    return nc


KERNEL_VARIANT = "tile"  # "tile" or "raw"


def kernel(query=None, key=None, value=None, q_param=None, _trace=False):
    from concourse.bass_utils import run_bass_kernel_spmd

    global LAST_RESULT

    value = np.ascontiguousarray(np.asarray(value, dtype=np.float32))
    assert value.shape == (B, SK, DV), value.shape

    nc = _build_nc_raw() if KERNEL_VARIANT == "raw" else _build_nc()
    shards = value.reshape(N_CORES, BPC, SK, DV)
    in_maps = [{"value": shards[i]} for i in range(N_CORES)]

    LAST_RESULT = run_bass_kernel_spmd(
        nc, in_maps, list(range(N_CORES)), trace=_trace
    )
    return np.concatenate(
        [LAST_RESULT.results[i]["out"] for i in range(N_CORES)], axis=0
    )


# revision 26
# speedup vs baseline: 1.0394x; 1.0394x over previous
"""Trainium2 Bass kernel for nn_Attention_39676907884025.

Reference semantics: q_param (a scalar) is broadcast over both query and key,
so the score matrix qk[b,q,k] = sum_d p*p is CONSTANT along the softmax axis.
Softmax of a constant row is exactly uniform (x - max(x) == 0 bit-exactly,
exp(0) == 1, sum == SK exactly, 1/SK is a power of two), so

    out[b, q, :] = (1/SK) * sum_k value[b, k, :]     for every q.

query / key / q_param never need to touch the device.

Distribution: data-parallel over batch B=16 across 8 NeuronCores (2 batches
per core). Per core and batch (Tile-scheduled, measured 28.3us on HW):
  1. load value[b] (2048, 128) in four 256KB quarter-chunks, alternating the
     two HWDGE queues (Sync/Act); SBUF layout xt[p, t*128+d] = V[p*16+t, d]
     so each partition reads 16 consecutive DRAM rows (contiguous runs),
  2. exact fp32 DVE add-tree per quarter as it lands (overlaps later loads),
     then combine to acc (128, 128),
  3. one fp32 matmul with a constant (1/2048) matrix as stationary weights:
     psum[q, d] = sum_p (1/SK) * acc[p, d] -- reduces across partitions AND
     broadcasts the softmax-weighted mean row to all 128 q-partitions,
  4. replicate the tile 4x along free (doubling copies), then four 256KB
     stores (alternating queues) covering 4 output row-tiles each.
"""

import sys

import numpy as np

if "/opt/trn_rl_repo" not in sys.path:
    sys.path.insert(0, "/opt/trn_rl_repo")

B, SQ, SK, D, DV = 16, 2048, 2048, 128, 128
N_CORES = 8
BPC = B // N_CORES  # batches per core
P = 128

LAST_RESULT = None  # BassKernelResults of the most recent run (for profiling)


def _build_nc():
    import concourse.bacc as bacc
    import concourse.mybir as mybir
    from concourse.tile import TileContext

    f32 = mybir.dt.float32
    nc = bacc.Bacc("TRN2", target_bir_lowering=False)

    val = nc.dram_tensor("value", [BPC, SK, DV], f32, kind="ExternalInput")
    out = nc.dram_tensor("out", [BPC, SQ, DV], f32, kind="ExternalOutput")

    nt = SK // P  # 16 k-tiles per batch
    nq = SQ // P  # 16 q-tiles per batch

    with TileContext(nc) as tc:
        with (
            tc.tile_pool(name="x", bufs=2) as xpool,
            tc.tile_pool(name="tree", bufs=2) as tpool,
            tc.tile_pool(name="const", bufs=1) as cpool,
            tc.tile_pool(name="psum", bufs=2, space="PSUM") as ppool,
        ):
            # Stationary matmul operand: every entry 1/SK (exact power of 2),
            # so the partition-reduction matmul also applies the softmax
            # weight exactly.
            w = cpool.tile([P, P], f32)
            nc.vector.memset(w[:], 1.0 / SK)

            # Queue pick per chunk index: even -> Sync HWDGE, odd -> Act HWDGE.
            dma_eng = [nc.sync, nc.scalar]

            for b in range(BPC):
                # SBUF xt[p, t*128 + d] = value[b, p*16 + t, d]: each
                # partition owns 16 consecutive DRAM rows (8KB contiguous).
                # Load in 4 quarter-chunks (256KB each, alternating HWDGE
                # queues) and reduce each quarter as soon as it lands, so
                # DVE work overlaps the remaining loads.
                xt = xpool.tile([P, SK], f32)
                xdst = xt[:].rearrange("p (t d) -> p t d", d=DV)
                xsrc = val[b].rearrange("(p t) d -> p t d", p=P)

                accs = []
                for qi in range(4):
                    t0, t1 = 4 * qi, 4 * (qi + 1)
                    dma_eng[qi % 2].dma_start(
                        xdst[:, t0:t1, :], xsrc[:, t0:t1, :]
                    )
                    lo, hi = 512 * qi, 512 * (qi + 1)
                    a = tpool.tile([P, 256], f32, tag=f"a{qi % 2}")
                    nc.vector.tensor_add(
                        a[:], xt[:, lo : lo + 256], xt[:, lo + 256 : hi]
                    )
                    acc = tpool.tile([P, P], f32, tag=f"acc{qi}")
                    nc.vector.tensor_add(acc[:], a[:, 0:128], a[:, 128:256])
                    accs.append(acc)

                s01 = tpool.tile([P, P], f32, tag="s01")
                nc.vector.tensor_add(s01[:], accs[0][:], accs[1][:])
                s23 = tpool.tile([P, P], f32, tag="s23")
                nc.vector.tensor_add(s23[:], accs[2][:], accs[3][:])
                t4 = tpool.tile([P, P], f32, tag="t4")
                nc.vector.tensor_add(t4[:], s01[:], s23[:])

                # psum[q, d] = sum_p (1/SK) * t4[p, d]  for all q rows.
                ps = ppool.tile([P, P], f32)
                nc.tensor.matmul(ps[:], w[:], t4[:], start=True, stop=True)

                # Replicate the mean tile 4x along the free axis; each of the
                # 4 stores (alternating queues) reads it, covering 4 output
                # row-tiles (256KB) apiece.
                wide = xpool.tile([P, 512], f32, tag="wide")
                nc.vector.tensor_copy(wide[:, 0:P], ps[:])
                nc.vector.tensor_copy(wide[:, P : 2 * P], wide[:, 0:P])
                nc.vector.tensor_copy(wide[:, 2 * P : 4 * P], wide[:, 0 : 2 * P])

                odst = out[b].rearrange("(p t) d -> p t d", p=P)
                wsrc = wide[:].rearrange("p (t d) -> p t d", d=DV)
                for qi in range(4):
                    t0, t1 = 4 * qi, 4 * (qi + 1)
                    dma_eng[qi % 2].dma_start(odst[:, t0:t1, :], wsrc)

    nc.compile()
    return nc


def _build_nc_raw():
    """Raw bacc version (no TileContext): manual semaphores, minimal
    framework overhead. Loads are chunked (4 t-tiles, 256KB apiece, two
    chunks per batch per HWDGE queue) and each chunk is tree-reduced on DVE
    as soon as it lands, so DVE overlaps the remaining loads. The partition
    reduction runs as two PSUM-accumulating (1/SK)-weighted matmuls, then a
    4x replicate and two 256KB stores per queue per batch."""
    import concourse.bacc as bacc
    import concourse.mybir as mybir

    f32 = mybir.dt.float32
    nc = bacc.Bacc("TRN2", target_bir_lowering=False)

    val = nc.dram_tensor("value", [BPC, SK, DV], f32, kind="ExternalInput")
    out = nc.dram_tensor("out", [BPC, SQ, DV], f32, kind="ExternalOutput")

    w = nc.alloc_sbuf_tensor("w_const", [P, P], f32)
    warm = nc.alloc_sbuf_tensor("warm", [P, DV], f32)
    xts = [nc.alloc_sbuf_tensor(f"xt{b}", [P, SK], f32) for b in range(BPC)]
    t1s = [nc.alloc_sbuf_tensor(f"t1_{b}", [P, 1024], f32) for b in range(BPC)]
    t2s = [nc.alloc_sbuf_tensor(f"t2_{b}", [P, 512], f32) for b in range(BPC)]
    t3s = [nc.alloc_sbuf_tensor(f"t3_{b}", [P, 256], f32) for b in range(BPC)]
    t4s = [nc.alloc_sbuf_tensor(f"t4_{b}", [P, P], f32) for b in range(BPC)]
    wides = [nc.alloc_sbuf_tensor(f"wide{b}", [P, 512], f32) for b in range(BPC)]
    psA = [nc.alloc_psum_tensor(f"psA{b}", [P, P], f32) for b in range(BPC)]
    psB = [nc.alloc_psum_tensor(f"psB{b}", [P, P], f32) for b in range(BPC)]

    s_lq1 = nc.alloc_semaphore("s_lq1")
    s_lq2 = nc.alloc_semaphore("s_lq2")
    s_lq3 = nc.alloc_semaphore("s_lq3")
    s_w = nc.alloc_semaphore("s_w")
    s_dve = nc.alloc_semaphore("s_dve")
    s_mm = nc.alloc_semaphore("s_mm")
    s_wide = nc.alloc_semaphore("s_wide")
    s_sq1 = nc.alloc_semaphore("s_sq1")
    s_sq2 = nc.alloc_semaphore("s_sq2")
    s_warm = nc.alloc_semaphore("s_warm")

    def xdst(b):
        return xts[b][:].rearrange("p (t d) -> p t d", d=DV)

    def xsrc(b):
        return val[b].rearrange("(p t) d -> p t d", p=P)

    def odst(b):
        return out[b].rearrange("(p t) d -> p t d", p=P)

    def wsrc(b):
        return wides[b][:].rearrange("p (t d) -> p t d", d=DV)

    with nc.Block() as block:

        @block.sync
        def _(sync):
            # Tiny warm-up DMA so the queue's cold first-byte latency is
            # paid during the startup barrier, not the first real load.
            sync.dma_start(warm[0:1, :], val[0, 0:1, :]).then_inc(s_warm, 16)
            for b in range(BPC):  # front half: t-tiles 0..7 (512KB)
                sync.dma_start(
                    xdst(b)[:, 0:8, :], xsrc(b)[:, 0:8, :]
                ).then_inc(s_lq1, 16)
            # batch-0 front stores
            sync.wait_ge(s_wide, 1)
            sync.dma_start(odst(0)[:, 0:4, :], wsrc(0)).then_inc(s_sq1, 16)
            sync.dma_start(odst(0)[:, 4:8, :], wsrc(0)).then_inc(s_sq1, 16)
            sync.wait_ge(s_sq1, 64)
            sync.wait_ge(s_warm, 16)

        @block.scalar
        def _(scalar):
            # The Act HWDGE queue has a ~3us cold-start lag, so it only gets
            # work whose latency is hidden: batch-1's back-half load and all
            # of batch-1's stores.
            scalar.dma_start(warm[1:2, :], val[0, 1:2, :]).then_inc(s_warm, 16)
            scalar.dma_start(
                xdst(1)[:, 8:16, :], xsrc(1)[:, 8:16, :]
            ).then_inc(s_lq2, 16)
            scalar.wait_ge(s_wide, 2)
            for t0 in (0, 4, 8, 12):
                scalar.dma_start(
                    odst(1)[:, t0 : t0 + 4, :], wsrc(1)
                ).then_inc(s_sq2, 16)
            scalar.wait_ge(s_sq2, 64)
            scalar.wait_ge(s_warm, 32)

        @block.gpsimd
        def _(gpsimd):
            # Third DMA queue (SWDGE): batch-0 back-half load, batch-0 back
            # stores.
            gpsimd.dma_start(
                xdst(0)[:, 8:16, :], xsrc(0)[:, 8:16, :]
            ).then_inc(s_lq3, 16)
            gpsimd.wait_ge(s_wide, 1)
            gpsimd.dma_start(odst(0)[:, 8:12, :], wsrc(0)).then_inc(s_sq1, 16)
            gpsimd.dma_start(odst(0)[:, 12:16, :], wsrc(0)).then_inc(s_sq1, 16)
            gpsimd.wait_ge(s_sq1, 64)

        @block.vector
        def _(vector):
            vector.memset(w[:], 1.0 / SK).then_inc(s_w, 1)
            for b in range(BPC):
                xt, scr = xts[b], t1s[b]
                # front half (cols 0..1023): from sync
                vector.wait_ge(s_lq1, 16 * (b + 1))
                vector.tensor_add(
                    t2s[b][:], xt[:, 0:512], xt[:, 512:1024]
                )
                vector.tensor_add(
                    t3s[b][:], t2s[b][:, 0:256], t2s[b][:, 256:512]
                )
                vector.tensor_add(
                    scr[:, 0:128], t3s[b][:, 0:128], t3s[b][:, 128:256]
                ).then_inc(s_dve, 1)  # accF
                # back half (cols 1024..2047): b0 from gpsimd, b1 from scalar
                vector.wait_ge(s_lq3 if b == 0 else s_lq2, 16)
                vector.tensor_add(
                    scr[:, 128:640], xt[:, 1024:1536], xt[:, 1536:2048]
                )
                vector.tensor_add(
                    scr[:, 640:896], scr[:, 128:384], scr[:, 384:640]
                )
                vector.tensor_add(
                    scr[:, 896:1024], scr[:, 640:768], scr[:, 768:896]
                ).then_inc(s_dve, 1)  # accB
                # wide[0:128] = psA + psB (exact), then replicate to 4 copies.
                # (DVE reads at most one PSUM operand per op, so stage psA
                # into SBUF while mmB is still running.)
                wide = wides[b]
                vector.wait_ge(s_mm, 2 * b + 1)
                vector.tensor_copy(t2s[b][:, 0:P], psA[b][:])
                vector.wait_ge(s_mm, 2 * (b + 1))
                vector.tensor_add(wide[:, 0:P], t2s[b][:, 0:P], psB[b][:])
                vector.tensor_copy(wide[:, P : 2 * P], wide[:, 0:P])
                vector.tensor_copy(
                    wide[:, 2 * P : 4 * P], wide[:, 0 : 2 * P]
                ).then_inc(s_wide, 1)

        @block.tensor
        def _(tensor):
            tensor.wait_ge(s_w, 1)
            for b in range(BPC):
                tensor.wait_ge(s_dve, 2 * b + 1)
                nc.tensor.matmul(
                    psA[b][:], w[:], t1s[b][:, 0:128], start=True, stop=True
                ).then_inc(s_mm, 1)
                tensor.wait_ge(s_dve, 2 * b + 2)
                nc.tensor.matmul(
                    psB[b][:], w[:], t1s[b][:, 896:1024], start=True, stop=True
                ).then_inc(s_mm, 1)

    nc.compile()
    return nc


KERNEL_VARIANT = "tile"  # "tile" or "raw"


def kernel(query=None, key=None, value=None, q_param=None, _trace=False):
    from concourse.bass_utils import run_bass_kernel_spmd

    global LAST_RESULT

    value = np.ascontiguousarray(np.asarray(value, dtype=np.float32))
    assert value.shape == (B, SK, DV), value.shape

    nc = _build_nc_raw() if KERNEL_VARIANT == "raw" else _build_nc()
    shards = value.reshape(N_CORES, BPC, SK, DV)
    in_maps = [{"value": shards[i]} for i in range(N_CORES)]

    LAST_RESULT = run_bass_kernel_spmd(
        nc, in_maps, list(range(N_CORES)), trace=_trace
    )
    return np.concatenate(
        [LAST_RESULT.results[i]["out"] for i in range(N_CORES)], axis=0
    )


# revision 30
# speedup vs baseline: 1.1016x; 1.0598x over previous
"""Trainium2 Bass kernel for nn_Attention_39676907884025.

Reference semantics: q_param (a scalar) is broadcast over both query and key,
so the score matrix qk[b,q,k] = sum_d p*p is CONSTANT along the softmax axis.
Softmax of a constant row is exactly uniform (x - max(x) == 0 bit-exactly,
exp(0) == 1, sum == SK exactly, 1/SK is a power of two), so

    out[b, q, :] = (1/SK) * sum_k value[b, k, :]     for every q.

query / key / q_param never need to touch the device.

Distribution: data-parallel over batch B=16 across 8 NeuronCores (2 batches
per core). Per core and batch (Tile-scheduled, measured 28.3us on HW):
  1. load value[b] (2048, 128) in four 256KB quarter-chunks, alternating the
     two HWDGE queues (Sync/Act); SBUF layout xt[p, t*128+d] = V[p*16+t, d]
     so each partition reads 16 consecutive DRAM rows (contiguous runs),
  2. exact fp32 DVE add-tree per quarter as it lands (overlaps later loads),
     then combine to acc (128, 128),
  3. one fp32 matmul with a constant (1/2048) matrix as stationary weights:
     psum[q, d] = sum_p (1/SK) * acc[p, d] -- reduces across partitions AND
     broadcasts the softmax-weighted mean row to all 128 q-partitions,
  4. replicate the tile 4x along free (doubling copies), then four 256KB
     stores (alternating queues) covering 4 output row-tiles each.
"""

import sys

import numpy as np

if "/opt/trn_rl_repo" not in sys.path:
    sys.path.insert(0, "/opt/trn_rl_repo")

B, SQ, SK, D, DV = 16, 2048, 2048, 128, 128
N_CORES = 8
BPC = B // N_CORES  # batches per core
P = 128

LAST_RESULT = None  # BassKernelResults of the most recent run (for profiling)


def _build_nc():
    import concourse.bacc as bacc
    import concourse.mybir as mybir
    from concourse.tile import TileContext

    f32 = mybir.dt.float32
    nc = bacc.Bacc("TRN2", target_bir_lowering=False)

    val = nc.dram_tensor("value", [BPC, SK, DV], f32, kind="ExternalInput")
    out = nc.dram_tensor("out", [BPC, SQ, DV], f32, kind="ExternalOutput")

    nt = SK // P  # 16 k-tiles per batch
    nq = SQ // P  # 16 q-tiles per batch

    with TileContext(nc) as tc:
        with (
            tc.tile_pool(name="x", bufs=2) as xpool,
            tc.tile_pool(name="tree", bufs=2) as tpool,
            tc.tile_pool(name="const", bufs=1) as cpool,
            tc.tile_pool(name="psum", bufs=2, space="PSUM") as ppool,
        ):
            # Stationary matmul operand: every entry 1/SK (exact power of 2),
            # so the partition-reduction matmul also applies the softmax
            # weight exactly.
            w = cpool.tile([P, P], f32)
            nc.vector.memset(w[:], 1.0 / SK)

            # Queue pick per chunk index: even -> Sync HWDGE, odd -> Act HWDGE.
            dma_eng = [nc.sync, nc.scalar]

            for b in range(BPC):
                # SBUF xt[p, t*128 + d] = value[b, p*16 + t, d]: each
                # partition owns 16 consecutive DRAM rows (8KB contiguous).
                # Load in 4 quarter-chunks (256KB each, alternating HWDGE
                # queues) and reduce each quarter as soon as it lands, so
                # DVE work overlaps the remaining loads.
                xt = xpool.tile([P, SK], f32)
                xdst = xt[:].rearrange("p (t d) -> p t d", d=DV)
                xsrc = val[b].rearrange("(p t) d -> p t d", p=P)

                accs = []
                for qi in range(4):
                    t0, t1 = 4 * qi, 4 * (qi + 1)
                    dma_eng[qi % 2].dma_start(
                        xdst[:, t0:t1, :], xsrc[:, t0:t1, :]
                    )
                    lo, hi = 512 * qi, 512 * (qi + 1)
                    a = tpool.tile([P, 256], f32, tag=f"a{qi % 2}")
                    nc.vector.tensor_add(
                        a[:], xt[:, lo : lo + 256], xt[:, lo + 256 : hi]
                    )
                    acc = tpool.tile([P, P], f32, tag=f"acc{qi}")
                    nc.vector.tensor_add(acc[:], a[:, 0:128], a[:, 128:256])
                    accs.append(acc)

                s01 = tpool.tile([P, P], f32, tag="s01")
                nc.vector.tensor_add(s01[:], accs[0][:], accs[1][:])
                s23 = tpool.tile([P, P], f32, tag="s23")
                nc.vector.tensor_add(s23[:], accs[2][:], accs[3][:])
                t4 = tpool.tile([P, P], f32, tag="t4")
                nc.vector.tensor_add(t4[:], s01[:], s23[:])

                # psum[q, d] = sum_p (1/SK) * t4[p, d]  for all q rows.
                ps = ppool.tile([P, P], f32)
                nc.tensor.matmul(ps[:], w[:], t4[:], start=True, stop=True)

                # Replicate the mean tile 4x along the free axis; each of the
                # 4 stores (alternating queues) reads it, covering 4 output
                # row-tiles (256KB) apiece.
                wide = xpool.tile([P, 512], f32, tag="wide")
                nc.vector.tensor_copy(wide[:, 0:P], ps[:])
                nc.vector.tensor_copy(wide[:, P : 2 * P], wide[:, 0:P])
                nc.vector.tensor_copy(wide[:, 2 * P : 4 * P], wide[:, 0 : 2 * P])

                odst = out[b].rearrange("(p t) d -> p t d", p=P)
                wsrc = wide[:].rearrange("p (t d) -> p t d", d=DV)
                for qi in range(4):
                    t0, t1 = 4 * qi, 4 * (qi + 1)
                    dma_eng[qi % 2].dma_start(odst[:, t0:t1, :], wsrc)

    nc.compile()
    return nc


def _build_nc_raw():
    """Raw bacc version (no TileContext): manual semaphores, minimal
    framework overhead. Loads are chunked (4 t-tiles, 256KB apiece, two
    chunks per batch per HWDGE queue) and each chunk is tree-reduced on DVE
    as soon as it lands, so DVE overlaps the remaining loads. The partition
    reduction runs as two PSUM-accumulating (1/SK)-weighted matmuls, then a
    4x replicate and two 256KB stores per queue per batch."""
    import concourse.bacc as bacc
    import concourse.mybir as mybir

    f32 = mybir.dt.float32
    nc = bacc.Bacc("TRN2", target_bir_lowering=False)

    val = nc.dram_tensor("value", [BPC, SK, DV], f32, kind="ExternalInput")
    out = nc.dram_tensor("out", [BPC, SQ, DV], f32, kind="ExternalOutput")

    w = nc.alloc_sbuf_tensor("w_const", [P, P], f32)
    warm = nc.alloc_sbuf_tensor("warm", [P, DV], f32)
    xts = [nc.alloc_sbuf_tensor(f"xt{b}", [P, SK], f32) for b in range(BPC)]
    t1s = [nc.alloc_sbuf_tensor(f"t1_{b}", [P, 1024], f32) for b in range(BPC)]
    t2s = [nc.alloc_sbuf_tensor(f"t2_{b}", [P, 512], f32) for b in range(BPC)]
    t3s = [nc.alloc_sbuf_tensor(f"t3_{b}", [P, 256], f32) for b in range(BPC)]
    t4s = [nc.alloc_sbuf_tensor(f"t4_{b}", [P, P], f32) for b in range(BPC)]
    wides = [nc.alloc_sbuf_tensor(f"wide{b}", [P, 512], f32) for b in range(BPC)]
    psA = [nc.alloc_psum_tensor(f"psA{b}", [P, P], f32) for b in range(BPC)]
    psB = [nc.alloc_psum_tensor(f"psB{b}", [P, P], f32) for b in range(BPC)]

    s_lq1 = nc.alloc_semaphore("s_lq1")
    s_lq2 = nc.alloc_semaphore("s_lq2")
    s_lq3 = nc.alloc_semaphore("s_lq3")
    s_w = nc.alloc_semaphore("s_w")
    s_dve = nc.alloc_semaphore("s_dve")
    s_mm = nc.alloc_semaphore("s_mm")
    s_wide = nc.alloc_semaphore("s_wide")
    s_sq1 = nc.alloc_semaphore("s_sq1")
    s_sq2 = nc.alloc_semaphore("s_sq2")
    s_warm = nc.alloc_semaphore("s_warm")

    def xdst(b):
        return xts[b][:].rearrange("p (t d) -> p t d", d=DV)

    def xsrc(b):
        return val[b].rearrange("(p t) d -> p t d", p=P)

    def odst(b):
        return out[b].rearrange("(p t) d -> p t d", p=P)

    def wsrc(b):
        return wides[b][:].rearrange("p (t d) -> p t d", d=DV)

    with nc.Block() as block:

        @block.sync
        def _(sync):
            # Tiny warm-up DMA so the queue's cold first-byte latency is
            # paid during the startup barrier, not the first real load.
            sync.dma_start(warm[0:1, :], val[0, 0:1, :]).then_inc(s_warm, 16)
            for b in range(BPC):  # front half: t-tiles 0..7 (512KB)
                sync.dma_start(
                    xdst(b)[:, 0:8, :], xsrc(b)[:, 0:8, :]
                ).then_inc(s_lq1, 16)
            # batch-0 front stores
            sync.wait_ge(s_wide, 1)
            sync.dma_start(odst(0)[:, 0:4, :], wsrc(0)).then_inc(s_sq1, 16)
            sync.dma_start(odst(0)[:, 4:8, :], wsrc(0)).then_inc(s_sq1, 16)
            sync.wait_ge(s_sq1, 64)
            sync.wait_ge(s_warm, 16)

        @block.scalar
        def _(scalar):
            # The Act HWDGE queue has a ~3us cold-start lag, so it only gets
            # work whose latency is hidden: batch-1's back-half load and all
            # of batch-1's stores.
            scalar.dma_start(warm[1:2, :], val[0, 1:2, :]).then_inc(s_warm, 16)
            scalar.dma_start(
                xdst(1)[:, 8:16, :], xsrc(1)[:, 8:16, :]
            ).then_inc(s_lq2, 16)
            scalar.wait_ge(s_wide, 2)
            for t0 in (0, 4, 8, 12):
                scalar.dma_start(
                    odst(1)[:, t0 : t0 + 4, :], wsrc(1)
                ).then_inc(s_sq2, 16)
            scalar.wait_ge(s_sq2, 64)
            scalar.wait_ge(s_warm, 32)

        @block.gpsimd
        def _(gpsimd):
            # Third DMA queue (SWDGE): batch-0 back-half load, batch-0 back
            # stores.
            gpsimd.dma_start(
                xdst(0)[:, 8:16, :], xsrc(0)[:, 8:16, :]
            ).then_inc(s_lq3, 16)
            gpsimd.wait_ge(s_wide, 1)
            gpsimd.dma_start(odst(0)[:, 8:12, :], wsrc(0)).then_inc(s_sq1, 16)
            gpsimd.dma_start(odst(0)[:, 12:16, :], wsrc(0)).then_inc(s_sq1, 16)
            gpsimd.wait_ge(s_sq1, 64)

        @block.vector
        def _(vector):
            vector.memset(w[:], 1.0 / SK).then_inc(s_w, 1)
            for b in range(BPC):
                xt, scr = xts[b], t1s[b]
                # front half (cols 0..1023): from sync
                vector.wait_ge(s_lq1, 16 * (b + 1))
                vector.tensor_add(
                    t2s[b][:], xt[:, 0:512], xt[:, 512:1024]
                )
                vector.tensor_add(
                    t3s[b][:], t2s[b][:, 0:256], t2s[b][:, 256:512]
                )
                vector.tensor_add(
                    scr[:, 0:128], t3s[b][:, 0:128], t3s[b][:, 128:256]
                ).then_inc(s_dve, 1)  # accF
                # back half (cols 1024..2047): b0 from gpsimd, b1 from scalar
                vector.wait_ge(s_lq3 if b == 0 else s_lq2, 16)
                vector.tensor_add(
                    scr[:, 128:640], xt[:, 1024:1536], xt[:, 1536:2048]
                )
                vector.tensor_add(
                    scr[:, 640:896], scr[:, 128:384], scr[:, 384:640]
                )
                vector.tensor_add(
                    scr[:, 896:1024], scr[:, 640:768], scr[:, 768:896]
                ).then_inc(s_dve, 1)  # accB
                # wide[0:128] = psA + psB (exact), then replicate to 4 copies.
                # (DVE reads at most one PSUM operand per op, so stage psA
                # into SBUF while mmB is still running.)
                wide = wides[b]
                vector.wait_ge(s_mm, 2 * b + 1)
                vector.tensor_copy(t2s[b][:, 0:P], psA[b][:])
                vector.wait_ge(s_mm, 2 * (b + 1))
                vector.tensor_add(wide[:, 0:P], t2s[b][:, 0:P], psB[b][:])
                vector.tensor_copy(wide[:, P : 2 * P], wide[:, 0:P])
                vector.tensor_copy(
                    wide[:, 2 * P : 4 * P], wide[:, 0 : 2 * P]
                ).then_inc(s_wide, 1)

        @block.tensor
        def _(tensor):
            tensor.wait_ge(s_w, 1)
            for b in range(BPC):
                tensor.wait_ge(s_dve, 2 * b + 1)
                nc.tensor.matmul(
                    psA[b][:], w[:], t1s[b][:, 0:128], start=True, stop=True
                ).then_inc(s_mm, 1)
                tensor.wait_ge(s_dve, 2 * b + 2)
                nc.tensor.matmul(
                    psB[b][:], w[:], t1s[b][:, 896:1024], start=True, stop=True
                ).then_inc(s_mm, 1)

    nc.compile()
    return nc


KERNEL_VARIANT = "tile"  # "tile" or "raw"


def kernel(query=None, key=None, value=None, q_param=None, _trace=False):
    from concourse.bass_utils import run_bass_kernel_spmd

    global LAST_RESULT

    value = np.ascontiguousarray(np.asarray(value, dtype=np.float32))
    assert value.shape == (B, SK, DV), value.shape

    nc = _build_nc_raw() if KERNEL_VARIANT == "raw" else _build_nc()
    shards = value.reshape(N_CORES, BPC, SK, DV)
    in_maps = [{"value": shards[i]} for i in range(N_CORES)]

    LAST_RESULT = run_bass_kernel_spmd(
        nc, in_maps, list(range(N_CORES)), trace=_trace
    )
    return np.concatenate(
        [LAST_RESULT.results[i]["out"] for i in range(N_CORES)], axis=0
    )
